# revision 1
# baseline (speedup 1.0000x reference)
"""Deformable Conv2d (modulated, torchvision v2 layout) on 8 Trainium2 NeuronCores.

Strategy: data-parallel over batch (B=8 -> 1 image per core).
Per core, entirely on device, software-pipelined per 128-position block:
  1. offset/mask convs on TensorE (im2col-by-shifted-AP matmuls, bf16 in /
     fp32 psum), emitted in row-group chunks interleaved with the pipeline
     so the first gathers start ~18us in while later chunks still run.
  2. transpose conv output to position-major; compute bilinear corner
     weights (x mask) and int32 quad-table indices on VectorE. floor()
     uses the magic-number trick (exact in fp32 RNE, same on HW and sim).
  3. per (block, tap) indirect DMAs -- 128 single indices per instruction,
     the HW maximum -- gather 2KB bf16 "quad" rows (4 corners x 256 ch)
     from a zero-padded table, prefetched 3 blocks ahead.
  4. corner combine: per-partition-scalar products (corners A-C on DVE @4x
     SIMD, corner D on ACT via scale=AP) + a DVE add tree.
  5. TensorE transposes vals to channel-major (PSUM) -> ACT copies to
     SBUF; contraction matmul accumulates over taps in PSUM; outputs are
     staged via ACT and DMA'd to DRAM three blocks per transfer.
"""

import sys

sys.path.insert(0, "/opt/trn_rl_repo")

import numpy as np
import ml_dtypes

import concourse.bass as bass
import concourse.mybir as mybir
from concourse.bass_utils import run_bass_kernel_spmd
from concourse.tile import TileContext
from concourse.vector_clock import ScopedClock
from concourse.alu_op_type import AluOpType

F32 = mybir.dt.float32
BF16 = mybir.dt.bfloat16
I32 = mybir.dt.int32

K = 3
PAD = 1
H = W = 48
HW = H * W          # 2304
CIN = COUT = 256
NTAP = K * K        # 9
NPB = HW // 128     # 18 position blocks
TPAD = 2            # quad-table padding (pixels) on each side
TW = W + 2 * TPAD   # 52 table cols
TROWS = TW * TW     # 2704 table rows
QELEM = 4 * CIN     # 1024 bf16 per quad row (4 corners x 256 ch)
MAGIC = 12582912.0  # 1.5*2^23: (x + MAGIC) - MAGIC == rne(x), ulp=1 in [2^23, 2^24)


# ---------------------------------------------------------------------------
# TileContext patches for this walrus build:
#  - it accepts at most ONE sem-wait per instruction -> hoist extras onto nops
# ---------------------------------------------------------------------------

def _make_wait_nop(nc, engine, waits):
    inst = mybir.InstNoOp(name=nc.get_next_instruction_name(), ins=[], outs=[])
    inst.engine = engine
    inst.sync_info = mybir.SyncInfo(on_wait=list(waits), on_update=[])
    nc.register_instruction(inst)
    return inst


def split_excess_waits(nc, max_waits=1):
    for _bname, bbb in nc.bb_map.items():
        bb = bbb.bb
        changed = False
        new = []
        for inst in bb.instructions:
            si = inst.sync_info
            waits = list(si.on_wait or []) if si else []
            if len(waits) > max_waits:
                keep = waits[:max_waits]
                extra = waits[max_waits:]
                for i in range(0, len(extra), max_waits):
                    new.append(_make_wait_nop(nc, inst.engine, extra[i:i + max_waits]))
                si.on_wait = keep
                changed = True
            new.append(inst)
        if changed:
            bb.instructions = new


class PatchedTC(TileContext):
    def _drain_and_barrier(self, tick_clock, wait_clock):
        drain_inst = self.nc.sync.drain()
        wait_clock.add_sem_waits(
            drain_inst.ins, ScopedClock({None: tick_clock.global_clock})
        )
        self.nc.all_engine_barrier()
        popped = self.nc._tile_sem_poison_stack.pop()
        assert popped is self._sem_poison
        self.nc.clear_and_free_semaphores(list(self.sems.allocated().values()))
        self.nc.all_engine_barrier()

    def __exit__(self, *args):
        r = super().__exit__(*args)
        split_excess_waits(self.nc)
        return r


# ---------------------------------------------------------------------------
# Kernel builder
# ---------------------------------------------------------------------------

def build_kernel(stage="full"):
    nc = bass.Bass()

    xpad = nc.dram_tensor("xpad", [2, 128, 50 * 50], BF16, kind="ExternalInput")
    idf = nc.dram_tensor("idf", [128, 128], F32, kind="ExternalInput")
    idb = nc.dram_tensor("idb", [128, 128], BF16, kind="ExternalInput")
    xq = nc.dram_tensor("xq", [TROWS, QELEM], BF16, kind="ExternalInput")
    womT = nc.dram_tensor("womT", [NTAP, 2, 128, 27], BF16, kind="ExternalInput")
    ob = nc.dram_tensor("ob", [27, 1], F32, kind="ExternalInput")
    wcT = nc.dram_tensor("wcT", [NTAP, 2, 128, 256], BF16, kind="ExternalInput")
    byx = nc.dram_tensor("byx", [2, NPB, 128, NTAP], F32, kind="ExternalInput")

    if stage == "om":
        om_out = nc.dram_tensor("om_out", [NPB, 128, 27], F32, kind="ExternalOutput")
    elif stage == "idxw":
        idx_out = nc.dram_tensor("idx_out", [NPB, 128, NTAP], I32, kind="ExternalOutput")
        w_out = nc.dram_tensor("w_out", [NPB, 128, 4 * NTAP], F32, kind="ExternalOutput")
    else:
        out = nc.dram_tensor("out", [2, 128, HW], F32, kind="ExternalOutput")

    with PatchedTC(nc) as tc:
        import contextlib
        with contextlib.ExitStack() as ctx:
            _build_body(ctx, tc, nc, stage, locals())
    return nc


def _build_body(ctx, tc, nc, stage, T):
    xpad, xq, womT, ob, wcT, byx = (
        T["xpad"], T["xq"], T["womT"], T["ob"], T["wcT"], T["byx"],
    )

    const_pool = ctx.enter_context(tc.tile_pool(name="const", bufs=1))
    sb = ctx.enter_context(tc.tile_pool(name="sb", bufs=1))
    psum = ctx.enter_context(tc.tile_pool(name="psum", bufs=2, space="PSUM"))
    psum_t = ctx.enter_context(tc.tile_pool(name="psum_t", bufs=2, space="PSUM"))
    gpool = ctx.enter_context(tc.tile_pool(name="gath", bufs=3))
    ppool = ctx.enter_context(tc.tile_pool(name="prod", bufs=2))
    vpool = ctx.enter_context(tc.tile_pool(name="vals", bufs=2))
    tpsum = ctx.enter_context(tc.tile_pool(name="tpsum", bufs=2, space="PSUM"))
    cpool = ctx.enter_context(tc.tile_pool(name="contr", bufs=2, space="PSUM"))

    # --- constants / inputs to SBUF (xpad first: it gates the conv) ---
    xp_sb = const_pool.tile([128, 2, 50 * 50], BF16, tag="xp")
    for cb in range(2):
        nc.sync.dma_start(out=xp_sb[:, cb], in_=xpad[cb])

    wom_sb = const_pool.tile([128, NTAP, 2, 27], BF16)
    nc.sync.dma_start(out=wom_sb[:], in_=womT[:].rearrange("t c p m -> p t c m"))

    ob_sb = const_pool.tile([27, 1], F32)
    nc.sync.dma_start(out=ob_sb[:], in_=ob[:])

    ident = const_pool.tile([128, 128], F32)
    nc.sync.dma_start(out=ident[:], in_=T["idf"][:])
    ident_b = const_pool.tile([128, 128], BF16)
    nc.sync.dma_start(out=ident_b[:], in_=T["idb"][:])

    byx_sb = const_pool.tile([128, 2, NPB, NTAP], F32)
    nc.sync.dma_start(out=byx_sb[:], in_=byx[:].rearrange("a b p t -> p a b t"))

    wc_sb = const_pool.tile([128, NTAP, 2, 256], BF16)
    nc.sync.dma_start(out=wc_sb[:], in_=wcT[:].rearrange("t c p m -> p t c m"))

    # =====================================================================
    # Stage 1: offset/mask conv -> om_sb [27, HW] fp32 (by row group)
    # =====================================================================
    om_sb = sb.tile([27, HW], F32)
    row_groups = [(0, 4), (4, 4), (8, 10), (18, 10), (28, 10), (38, 10)]

    def emit_conv_rowgroup(r0, nr):
        ps = psum.tile([27, 480], F32, tag="omps")
        n = nr * W
        first = True
        for tap in range(NTAP):
            dy, dx = tap // K, tap % K
            for cb in range(2):
                rhs = xp_sb[:, cb].rearrange("p (h w) -> p h w", h=50)[
                    :, dy + r0:dy + r0 + nr, dx:dx + W
                ]
                nc.tensor.matmul(ps[:, :n], lhsT=wom_sb[:, tap, cb], rhs=rhs,
                                 start=first, stop=(tap == NTAP - 1 and cb == 1))
                first = False
        nc.scalar.activation(
            out=om_sb[:, r0 * W:r0 * W + n], in_=ps[:, :n],
            func=mybir.ActivationFunctionType.Identity,
            bias=ob_sb[:], scale=1.0,
        )

    # position-major conv output: om_t [128, NPB, 27]
    om_t = sb.tile([128, NPB, 27], F32)

    def emit_om_transpose(pb):
        pst = psum_t.tile([128, 27], F32, tag="omT")
        nc.tensor.transpose(out=pst[:], in_=om_sb[:, pb * 128:(pb + 1) * 128],
                            identity=ident[:27, :27])
        nc.vector.tensor_copy(out=om_t[:, pb], in_=pst[:])

    # =====================================================================
    # Stage 2: index + corner weight computation (sliced over pblock range)
    # =====================================================================
    shp = [128, NPB, NTAP]
    py = sb.tile(shp, F32)
    px = sb.tile(shp, F32)
    msk = sb.tile(shp, F32)
    ry = sb.tile(shp, F32)
    rx = sb.tile(shp, F32)
    iy = sb.tile(shp, I32)
    ix = sb.tile(shp, I32)
    fy = sb.tile(shp, F32)
    fx = sb.tile(shp, F32)
    qy = sb.tile(shp, I32)
    qx = sb.tile(shp, I32)
    qidx = sb.tile(shp, I32)
    gy = sb.tile(shp, F32)
    gx = sb.tile(shp, F32)
    u0 = sb.tile(shp, F32)
    u1 = sb.tile(shp, F32)
    wq = sb.tile([128, NPB, 4, NTAP], F32)

    def emit_stage2(lo, hi):
        s = slice(lo, hi)
        off_y = om_t[:, s, 0:18:2]
        off_x = om_t[:, s, 1:18:2]
        msk_l = om_t[:, s, 18:27]
        A = AluOpType
        V = nc.vector
        nc.scalar.activation(out=msk[:, s], in_=msk_l,
                             func=mybir.ActivationFunctionType.Sigmoid)
        V.tensor_tensor(out=py[:, s], in0=off_y, in1=byx_sb[:, 0, s], op=A.add)
        V.tensor_tensor(out=px[:, s], in0=off_x, in1=byx_sb[:, 1, s], op=A.add)
        # floor: r = rne(p - 0.5) (exact magic-number rounding)
        V.tensor_scalar(out=ry[:, s], in0=py[:, s], scalar1=-0.5, scalar2=MAGIC,
                        op0=A.add, op1=A.add)
        V.tensor_scalar(out=ry[:, s], in0=ry[:, s], scalar1=-MAGIC, scalar2=None,
                        op0=A.add)
        V.tensor_scalar(out=rx[:, s], in0=px[:, s], scalar1=-0.5, scalar2=MAGIC,
                        op0=A.add, op1=A.add)
        V.tensor_scalar(out=rx[:, s], in0=rx[:, s], scalar1=-MAGIC, scalar2=None,
                        op0=A.add)
        V.tensor_copy(out=iy[:, s], in_=ry[:, s])
        V.tensor_copy(out=ix[:, s], in_=rx[:, s])
        V.tensor_tensor(out=fy[:, s], in0=py[:, s], in1=ry[:, s], op=A.subtract)
        V.tensor_tensor(out=fx[:, s], in0=px[:, s], in1=rx[:, s], op=A.subtract)
        # qidx = clamp(iy+TPAD, 0, TW-1)*TW + clamp(ix+TPAD, 0, TW-1)
        V.tensor_scalar(out=qy[:, s], in0=iy[:, s], scalar1=TPAD, scalar2=0,
                        op0=A.add, op1=A.max)
        V.tensor_scalar(out=qy[:, s], in0=qy[:, s], scalar1=TW - 1, scalar2=TW,
                        op0=A.min, op1=A.mult)
        V.tensor_scalar(out=qx[:, s], in0=ix[:, s], scalar1=TPAD, scalar2=0,
                        op0=A.add, op1=A.max)
        V.tensor_scalar(out=qx[:, s], in0=qx[:, s], scalar1=TW - 1, scalar2=None,
                        op0=A.min)
        V.tensor_tensor(out=qidx[:, s], in0=qy[:, s], in1=qx[:, s], op=A.add)
        # corner weights: A=(1-fy)(1-fx)m B=(1-fy)fx m C=fy(1-fx)m D=fy fx m
        V.tensor_scalar(out=gy[:, s], in0=fy[:, s], scalar1=-1.0, scalar2=1.0,
                        op0=A.mult, op1=A.add)
        V.tensor_scalar(out=gx[:, s], in0=fx[:, s], scalar1=-1.0, scalar2=1.0,
                        op0=A.mult, op1=A.add)
        V.tensor_tensor(out=u0[:, s], in0=gy[:, s], in1=msk[:, s], op=A.mult)
        V.tensor_tensor(out=u1[:, s], in0=fy[:, s], in1=msk[:, s], op=A.mult)
        V.tensor_tensor(out=wq[:, s, 0], in0=u0[:, s], in1=gx[:, s], op=A.mult)
        V.tensor_tensor(out=wq[:, s, 1], in0=u0[:, s], in1=fx[:, s], op=A.mult)
        V.tensor_tensor(out=wq[:, s, 2], in0=u1[:, s], in1=gx[:, s], op=A.mult)
        V.tensor_tensor(out=wq[:, s, 3], in0=u1[:, s], in1=fx[:, s], op=A.mult)

    # --- emit stage 1/2 finely interleaved so the first gathers start early:
    # after each conv row group, transpose + stage2 for the pblocks it
    # completes.  Row limits: pblock pb needs conv rows <= (pb*128+127)//48.
    s12_plan = [  # (rowgroup_idx, om-transpose/stage2 pblock range)
        (0, 0, 1), (1, 1, 3), (2, 3, 6), (3, 6, 10), (4, 10, 14), (5, 14, 18),
    ]

    def emit_s12_chunk(ci):
        rg, lo, hi = s12_plan[ci]
        emit_conv_rowgroup(*row_groups[rg])
        for pb in range(lo, hi):
            emit_om_transpose(pb)
        emit_stage2(lo, hi)

    if stage == "om" or stage == "idxw":
        for ci in range(5):
            emit_s12_chunk(ci)
        if stage == "om":
            nc.sync.dma_start(out=T["om_out"][:].rearrange("b p m -> p b m"),
                              in_=om_t[:])
        else:
            nc.sync.dma_start(out=T["idx_out"][:].rearrange("b p t -> p b t"),
                              in_=qidx[:])
            nc.sync.dma_start(out=T["w_out"][:].rearrange("b p m -> p b m"),
                              in_=wq[:].rearrange("p b c t -> p b (c t)"))
        return

    out_dram = T["out"]

    # =====================================================================
    # Stage 3-5 pipeline, one 128-position block per iteration
    # =====================================================================
    def emit_gather(pb):
        g = gpool.tile([128, NTAP, QELEM], BF16, tag="g")
        for t in range(NTAP):
            nc.gpsimd.indirect_dma_start(
                out=g[:, t], out_offset=None, in_=xq[:],
                in_offset=bass.IndirectOffsetOnAxis(ap=qidx[:, pb, t:t + 1], axis=0),
            )
        return g

    emit_s12_chunk(0)
    g_tiles = {0: emit_gather(0)}
    emit_s12_chunk(1)
    g_tiles[1] = emit_gather(1)
    g_tiles[2] = emit_gather(2)
    emit_s12_chunk(2)
    emit_s12_chunk(3)
    emit_s12_chunk(4)
    emit_s12_chunk(5)
    s12_at = {}

    OTB = 3  # output pblocks batched per DMA
    ot = None
    for pb in range(NPB):
        if pb + 3 < NPB:
            g_tiles[pb + 3] = emit_gather(pb + 3)
        if pb in s12_at:
            emit_s12_chunk(s12_at[pb])
        g = g_tiles.pop(pb)
        # --- corner products -> P (DVE @4x), tree adds (DVE), v = final sum
        P = ppool.tile([128, NTAP, 4, 256], BF16, tag="P")
        for t in range(NTAP):
            for j in range(4):
                if j == 3:
                    # corner D products ride the ACT engine (per-partition
                    # scale AP), taking ~1/4 of the combine off DVE
                    nc.scalar.activation(
                        out=P[:, t, j], in_=g[:, t, j * 256:(j + 1) * 256],
                        func=mybir.ActivationFunctionType.Copy,
                        scale=wq[:, pb, j, t:t + 1])
                else:
                    nc.vector.tensor_scalar(
                        out=P[:, t, j], in0=g[:, t, j * 256:(j + 1) * 256],
                        scalar1=wq[:, pb, j, t:t + 1],
                        scalar2=None, op0=AluOpType.mult)
        nc.vector.tensor_tensor(out=P[:, :, 0], in0=P[:, :, 0],
                                in1=P[:, :, 1], op=AluOpType.add)
        nc.vector.tensor_tensor(out=P[:, :, 2], in0=P[:, :, 2],
                                in1=P[:, :, 3], op=AluOpType.add)
        v = vpool.tile([128, NTAP, 256], BF16, tag="v")
        nc.vector.tensor_tensor(out=v[:], in0=P[:, :, 0],
                                in1=P[:, :, 2], op=AluOpType.add)

        # --- transpose to channel-major: 18 tiles in 3 psum groups
        vt = vpool.tile([128, 2 * NTAP, 128], BF16, tag="vt")
        for grp in range(3):
            pst = tpsum.tile([128, 6, 128], BF16, tag="T")
            for i in range(6):
                kk = grp * 6 + i
                t, cb = kk // 2, kk % 2
                nc.tensor.transpose(out=pst[:, i],
                                    in_=v[:, t, cb * 128:(cb + 1) * 128],
                                    identity=ident_b[:])
            nc.scalar.activation(out=vt[:, grp * 6:(grp + 1) * 6], in_=pst[:],
                                 func=mybir.ActivationFunctionType.Copy)

        # --- contraction: psum[o, p] = sum_{t, cb} wcT.T @ vt
        cps = cpool.tile([128, 2, 128], F32, tag="c")
        for ob_i in range(2):
            for kk in range(2 * NTAP):
                t, cb = kk // 2, kk % 2
                nc.tensor.matmul(
                    cps[:, ob_i],
                    lhsT=wc_sb[:, t, cb, 128 * ob_i:128 * (ob_i + 1)],
                    rhs=vt[:, kk],
                    start=(kk == 0), stop=(kk == 2 * NTAP - 1),
                )
        # --- output: PSUM -> SBUF staging (ACT), DMA every OTB blocks
        # (GPSIMD cannot access PSUM on HW)
        if pb % OTB == 0:
            ot = vpool.tile([128, 2, OTB, 128], F32, tag="ot")
        nc.scalar.activation(out=ot[:, :, pb % OTB], in_=cps[:],
                             func=mybir.ActivationFunctionType.Copy)
        if pb % OTB == OTB - 1:
            nc.sync.dma_start(
                out=out_dram[:, :, (pb - OTB + 1) * 128:(pb + 1) * 128]
                    .rearrange("a p m -> p a m"),
                in_=ot[:])


# ---------------------------------------------------------------------------
# Host-side wrapper
# ---------------------------------------------------------------------------

def _prep_core_inputs(xb, weight, off_w, off_b, mask_w, mask_b):
    """Build per-core input dict for one image xb [256, 48, 48] fp32."""
    xb_bf = xb.astype(ml_dtypes.bfloat16)
    xpad = np.zeros((256, 50, 50), ml_dtypes.bfloat16)
    xpad[:, 1:49, 1:49] = xb_bf
    xpad2 = xpad.reshape(2, 128, 50 * 50)

    # quad table: rows keyed (y+TPAD)*TW + (x+TPAD), y,x in [-TPAD, 48+TPAD)
    ext = np.zeros((256, TW + 1, TW + 1), ml_dtypes.bfloat16)
    ext[:, TPAD:TPAD + 48, TPAD:TPAD + 48] = xb_bf
    q = np.zeros((TW, TW, 4, 256), ml_dtypes.bfloat16)
    q[:, :, 0] = ext[:, :TW, :TW].transpose(1, 2, 0)
    q[:, :, 1] = ext[:, :TW, 1:TW + 1].transpose(1, 2, 0)
    q[:, :, 2] = ext[:, 1:TW + 1, :TW].transpose(1, 2, 0)
    q[:, :, 3] = ext[:, 1:TW + 1, 1:TW + 1].transpose(1, 2, 0)
    xq = q.reshape(TROWS, QELEM)

    wom = np.concatenate([off_w, mask_w], axis=0)          # [27, 256, 3, 3]
    womT = wom.reshape(27, 2, 128, K * K).transpose(3, 1, 2, 0).astype(
        ml_dtypes.bfloat16).copy()
    ob = np.concatenate([off_b, mask_b])[:, None].astype(np.float32)

    wcT = weight.reshape(256, 2, 128, K * K).transpose(3, 1, 2, 0).astype(
        ml_dtypes.bfloat16).copy()

    p = np.arange(HW)
    oy, ox = p // W, p % W
    ky, kx = np.meshgrid(np.arange(K), np.arange(K), indexing="ij")
    BY = (oy[:, None] + ky.reshape(-1)[None, :] - 1).astype(np.float32)
    BX = (ox[:, None] + kx.reshape(-1)[None, :] - 1).astype(np.float32)
    byx = np.stack([BY, BX]).reshape(2, NPB, 128, NTAP)

    idf = np.eye(128, dtype=np.float32)
    idb = np.eye(128, dtype=np.float32).astype(ml_dtypes.bfloat16)

    return dict(xpad=xpad2, xq=xq, womT=womT, ob=ob, wcT=wcT, byx=byx,
                idf=idf, idb=idb)


_CACHED = {}


def kernel(x, weight, off_w, off_b, mask_w, mask_b, _stage="full", _trace=False):
    x = np.asarray(x, np.float32)
    weight = np.asarray(weight, np.float32)
    off_w = np.asarray(off_w, np.float32)
    off_b = np.asarray(off_b, np.float32)
    mask_w = np.asarray(mask_w, np.float32)
    mask_b = np.asarray(mask_b, np.float32)
    B = x.shape[0]
    assert B == 8

    if _stage not in _CACHED:
        _CACHED[_stage] = build_kernel(_stage)
    nc = _CACHED[_stage]

    in_maps = [
        _prep_core_inputs(x[b], weight, off_w, off_b, mask_w, mask_b)
        for b in range(B)
    ]
    try:
        res = run_bass_kernel_spmd(nc, in_maps, core_ids=list(range(8)), trace=_trace)
    except (ImportError, ModuleNotFoundError):
        res = run_bass_kernel_spmd(nc, in_maps, core_ids=list(range(8)), trace=False)
    if _stage != "full":
        return res

    out = np.empty((B, COUT, H, W), np.float32)
    for b in range(B):
        o = res.results[b]["out"]           # [2, 128, HW]
        out[b] = o.reshape(COUT, H, W)
    kernel._last_exec_time_ns = res.exec_time_ns
    return out

